# revision 13
# baseline (speedup 1.0000x reference)
"""Contrast-depth MSE loss on 8 Trainium2 NeuronCores.

Math: with d = out - label (per image, 32x32 grid flattened to p in [0,1024)),
the loss is an exact quadratic form

    loss = sum_{p,q} C[p,q] * G[p,q] / (B*8*30*30),
    G[p,q] = sum_img d[img,p] * d[img,q]

where C (the contrast-depth-conv quadratic form) is supported on the
diagonals q-p in {0, +-1, +-31, +-32, +-33}.  Each core computes banded
Gram blocks G[128k+r, 128k+c] (c in [0,161)) on the TensorEngine with
PSUM accumulation over its 2048-image shard; the host applies the C
weights to the diagonals and reduces across cores.

Scheduling: HWDGE splits a [p0:p1, :] DMA across the largest divisor of
(p1-p0) that is <= 16 SDMA engines, assigned positionally from engine 0.
SDMA engine 15 is ~20% slower than the rest, so the layout is tilted:
13 image-slots are full-width (engine 15 carries exactly those), 3 extra
slots ride on [0:120] chunks that split across engines 0-14 only, and a
last 24-image slot goes on a [0:24] chunk (engines 0-11).  All engines
then finish the stream together.  The host packs each shard
partition-major per chunk so every DMA reads long contiguous runs.
Chunks are ramped (small first chunk for an early vector start, 2MB
mid-stream, 0.5MB single-slot chunks at the end) so the tail after the
final byte is one split subtract + 8 matmuls overlapped with PSUM->SBUF
copies on the ACT and DVE engines, then 2 output DMA slices.
"""

import numpy as np

_B = 16384
_H = 32
_W = 32
_P = _H * _W  # 1024 pixels
_NCORES = 8
_BSH = _B // _NCORES  # 2048 images per core
_TILE = 128
_BAND = 161  # 128 + max diagonal offset (33)
_NSLOT = 17  # max image-slots per partition
_FREE = _NSLOT * _P


def _block_ncols(k: int) -> int:
    return min(_BAND, _P - 128 * k)


_GRAM_COLS = sum(_block_ncols(k) for k in range(8))  # 7*161 + 128 = 1255


def _build_weights() -> np.ndarray:
    """[128, _GRAM_COLS] weights s.t. loss_sum = sum(W * gram_blocks)."""
    C = np.zeros((_P, _P), dtype=np.float64)
    offs = [(a, b) for a in range(3) for b in range(3) if (a, b) != (1, 1)]
    for a, b in offs:
        for i in range(_H - 2):
            for j in range(_W - 2):
                p = (i + a) * _W + (j + b)  # neighbor pixel
                q = (i + 1) * _W + (j + 1)  # center pixel
                C[p, p] += 1.0
                C[q, q] += 1.0
                C[p, q] -= 1.0
                C[q, p] -= 1.0
    W = np.zeros((_TILE, _GRAM_COLS), dtype=np.float64)
    off = 0
    for k in range(8):
        ncols = _block_ncols(k)
        for delta in (0, 1, 31, 32, 33):
            for r in range(_TILE):
                p = 128 * k + r
                q = p + delta
                c = r + delta
                if q >= _P or c >= ncols:
                    continue
                W[r, off + c] = C[p, q] * (1.0 if delta == 0 else 2.0)
        off += ncols
    return W


_WFULL = _build_weights()

# chunk table in DMA order: (npart, slot0, nslots).  Slots 0-12 exist on
# all 128 partitions, 13-15 on partitions [0,120), 16 on [0,24).
# 13*128 + 24 + 3*120 = 2048 images.  Each chunk reads a contiguous
# run of input rows (partition p <- rows [base + p*ns, base + (p+1)*ns)
# via the DMA's row->partition reshape), so every SDMA engine's
# descriptor set is a contiguous DRAM block and no host packing is
# needed -- any image-to-slot assignment is valid since the Gram sums
# over images.
_CHUNKS = [
    (128, 0, 1),
    (128, 1, 4),
    (128, 5, 4),
    (128, 9, 4),
    (24, 16, 1),
    (120, 13, 1),
    (120, 14, 1),
    (120, 15, 1),
]
assert sum(np * ns for np, _, ns in _CHUNKS) == _BSH

# per-slot compute order (the last one is the tail tile); (slot, npart)
_SLOT_ORDER = (
    [(0, 128)]
    + [(s, 128) for s in range(1, 13)]
    + [(16, 24), (13, 120), (14, 120), (15, 120)]
)

_NC_CACHE = None


def _build_nc():
    import concourse.bacc as bacc
    import concourse.mybir as mybir
    import concourse.tile as tile

    nc = bacc.Bacc()
    out_d = nc.dram_tensor("out", [_BSH, _P], mybir.dt.float32, kind="ExternalInput")
    lab_d = nc.dram_tensor("label", [_BSH, _P], mybir.dt.float32, kind="ExternalInput")
    gram_d = nc.dram_tensor(
        "gram", [_TILE, _GRAM_COLS], mybir.dt.float32, kind="ExternalOutput"
    )

    with tile.TileContext(nc) as tc:
        with (
            tc.tile_pool(name="buf", bufs=1) as buf_pool,
            tc.tile_pool(name="ps", bufs=1, space="PSUM") as psum_pool,
        ):
            grams = []
            offs = []
            off = 0
            for k in range(8):
                ncols = _block_ncols(k)
                grams.append(
                    psum_pool.tile(
                        [_TILE, ncols], mybir.dt.float32, tag=f"g{k}", name=f"g{k}"
                    )
                )
                offs.append(off)
                off += ncols

            # persistent SBUF buffers: every chunk DMA can enqueue
            # immediately; no pool-slot rotation ever blocks the DMA stream.
            o = buf_pool.tile([_TILE, _FREE], mybir.dt.float32, tag="o", name="o")
            lb = buf_pool.tile([_TILE, _FREE], mybir.dt.float32, tag="l", name="l")
            d = buf_pool.tile([_TILE, _FREE], mybir.dt.bfloat16, tag="d", name="d")
            result = buf_pool.tile(
                [_TILE, _GRAM_COLS], mybir.dt.float32, tag="r", name="r"
            )

            base = 0
            for npart, s0, ns in _CHUNKS:
                c0, c1 = s0 * _P, (s0 + ns) * _P
                n = npart * ns
                nc.sync.dma_start(out=o[0:npart, c0:c1], in_=out_d[base : base + n, :])
                nc.scalar.dma_start(
                    out=lb[0:npart, c0:c1], in_=lab_d[base : base + n, :]
                )
                base += n

            def emit_mms(slot, npart, start, stop):
                c0 = slot * _P
                for k in range(8):
                    ncols = _block_ncols(k)
                    nc.tensor.matmul(
                        grams[k][:, :ncols],
                        lhsT=d[0:npart, c0 + 128 * k : c0 + 128 * k + 128],
                        rhs=d[0:npart, c0 + 128 * k : c0 + 128 * k + ncols],
                        start=start,
                        stop=stop,
                    )

            nslots = len(_SLOT_ORDER)
            for si, (s, npart) in enumerate(_SLOT_ORDER):
                c0, c1 = s * _P, (s + 1) * _P
                if si < nslots - 1:
                    nc.vector.tensor_sub(
                        out=d[0:npart, c0:c1], in0=o[0:npart, c0:c1], in1=lb[0:npart, c0:c1]
                    )
                    emit_mms(s, npart, start=(si == 0), stop=False)
                else:
                    # tail slot: split the subtract at the block 4/5
                    # boundary so matmuls and PSUM->SBUF copies start
                    # before it finishes.
                    sp = 673
                    nc.vector.tensor_sub(
                        out=d[0:npart, c0 : c0 + sp],
                        in0=o[0:npart, c0 : c0 + sp],
                        in1=lb[0:npart, c0 : c0 + sp],
                    )
                    nc.vector.tensor_sub(
                        out=d[0:npart, c0 + sp : c1],
                        in0=o[0:npart, c0 + sp : c1],
                        in1=lb[0:npart, c0 + sp : c1],
                    )
                    emit_mms(s, npart, start=False, stop=True)

            # PSUM -> SBUF: blocks 0-3 on the ACT engine, 4-7 on DVE (which
            # is finishing the tail subtract), then 2 output DMA slices.
            for k in range(8):
                ncols = _block_ncols(k)
                dst = result[:, offs[k] : offs[k] + ncols]
                if k < 4:
                    nc.scalar.copy(out=dst, in_=grams[k][:])
                else:
                    nc.vector.tensor_copy(out=dst, in_=grams[k][:])
            split = offs[4]
            nc.sync.dma_start(out=gram_d[:, :split], in_=result[:, :split])
            nc.sync.dma_start(out=gram_d[:, split:], in_=result[:, split:])
    nc.finalize()
    return nc


def _run(out, label, trace=False):
    from concourse.bass_utils import run_bass_kernel_spmd

    global _NC_CACHE
    out = np.ascontiguousarray(np.asarray(out), dtype=np.float32).reshape(_B, _P)
    label = np.ascontiguousarray(np.asarray(label), dtype=np.float32).reshape(_B, _P)
    if _NC_CACHE is None:
        _NC_CACHE = _build_nc()
    in_maps = [
        {
            "out": out[i * _BSH : (i + 1) * _BSH],
            "label": label[i * _BSH : (i + 1) * _BSH],
        }
        for i in range(_NCORES)
    ]
    res = run_bass_kernel_spmd(
        _NC_CACHE, in_maps, core_ids=list(range(_NCORES)), trace=trace
    )
    total = 0.0
    for r in res.results:
        total += float((_WFULL * r["gram"].astype(np.float64)).sum())
    loss = total / (_B * 8 * (_H - 2) * (_W - 2))
    return np.asarray(np.float32(loss)), res


def kernel(out, label):
    loss, _ = _run(out, label, trace=False)
    return loss
